# revision 1
# baseline (speedup 1.0000x reference)
"""Memory-efficient Dice loss on 8 Trainium2 NeuronCores.

Full inputs:
  logits  (2, 16, 64, 128, 128) fp32
  targets (2, 64, 128, 128) int64  (values 0..15)
Output: scalar fp32 loss = 1 - mean_{b, c != 0} dice[b, c].

Sharding: 8 cores over (B=2) x (D quartered into 4 slabs of 16).
Each core reduces its shard to a single 119x119 stats matrix; host
combines the tiny per-core stats and applies the dice formula.

Per-core math (voxels n, classes c):
  e[n,c]   = exp(logit[n,c])            (no max-sub needed; |logit| < ~6)
  Z[n]     = sum_c e[n,c]
  r[n]     = 1/Z[n]
  mr[n,c'] = (t[n] == c') * r[n]
  Stats via a PSUM-accumulated matmul contracting over voxels:
    lhsT = [e (16 cols) | Z],  rhs = [mr (16 cols) | r]
    out[c,c']   -> diag = intersection[c] = sum_n prob[n,c]*(t==c)
    out[c,16]   -> probs_sum[c] = sum_n prob[n,c]
    out[16,c']  -> counts[c']   = sum_n Z*r*(t==c') == sum_n (t==c')

DMA design (measured on HW): per-DMA fixed cost ~0.9us serializes per
HWDGE ring, and any AP whose per-partition stream hops at the 1 MiB
class pitch collapses HBM bandwidth ~5x (bank aliasing). So each
dma_start moves one CONTIGUOUS (class, voxel-block) region, blocks are
pipelined, and logits DMAs alternate between the two HWDGE rings
(nc.sync / nc.scalar) to halve the serialized fixed cost.

Engine split per compute sub-iteration (DVE drain tax makes big DVE ops
~2x cost, and GPSIMD runs concurrently since all DVE ops here are 1x):
  ACT   : exp (also converts class-major -> chunk-major layout)
  GPSIMD: Z-tree levels 1-2 (big adds, no DVE drain tax on Q7)
  DVE   : Z-tree tail, 1/Z, 16 per-class (t==c)*r ops (small, drain-free)
  PE    : stats matmuls, PSUM-accumulated
"""

import numpy as np

import concourse.bass as bass
import concourse.mybir as mybir
import concourse.tile as tile
from concourse import bacc
from concourse.bass_utils import run_bass_kernel_spmd

B, C, D, H, W = 2, 16, 64, 128, 128
P = 128            # SBUF partitions
NCORES = 8
DSH = D // 4       # d-planes per core
N = DSH * H * W    # voxels per core = 262144
M17 = C + 1        # 17 = classes + (Z | r) slot
G = 7              # packed chunk-columns per matmul
MOUT = G * M17     # 119

SMOOTH = 1.0
IGNORE_INDEX = 0


def build(n_vox=N, nblk=4, tsub=128, loop_reps=1, fast_recip=True, stages=None):
    """Build the SPMD single-core Bass program.

    n_vox = P * nblk * BW voxels; BW per-partition elements per block;
    compute consumes each block in sub-iterations of tsub columns.
    stages: None for the full kernel, or a cumulative subset of
    {"act", "gp", "dvez", "recip", "stt", "mm"} for HW bisection.
    """
    assert n_vox % (P * nblk) == 0
    BW = n_vox // (P * nblk)
    tsub = min(tsub, BW)
    assert BW % tsub == 0
    nsub = BW // tsub
    T = tsub
    full = stages is None
    stages = stages or set()

    def on(s):
        return full or s in stages

    fp32 = mybir.dt.float32
    AL = mybir.AluOpType

    nc = bacc.Bacc("TRN2", target_bir_lowering=False, debug=False)
    logits_d = nc.dram_tensor("logits", [C, n_vox], fp32, kind="ExternalInput")
    # int64 targets are passed as int32 pairs (jax x64-off canonicalization
    # would otherwise silently truncate the input array to 4-byte elements)
    targets_d = nc.dram_tensor(
        "targets", [2 * n_vox], mybir.dt.int32, kind="ExternalInput"
    )
    out_d = nc.dram_tensor("out", [MOUT, MOUT], fp32, kind="ExternalOutput")

    # Block (sweep) b, class c: partition p reads run
    # [p*nblk*BW + b*BW, +BW) — the b-th slice of each partition's
    # full-pitch run. The full-pitch stride keeps the AP un-mergeable
    # (a merged fully-contiguous AP overflows the 16-bit ISA num_elem
    # field) while addresses stay ascending with 4 KiB-class descriptors.
    src_log = logits_d.ap().rearrange("c (p b j) -> c b p j", b=nblk, p=P)
    src_tgt = targets_d.ap().rearrange("(p b j k) -> b p j k", b=nblk, p=P, k=2)

    nmm = (T + G - 1) // G  # matmuls per sub-iteration

    def body(tc, pools):
        lpool, epool, rpool, zpool, small, psump, fin = pools
        acc = psump.tile([MOUT, MOUT], fp32)
        for blk in range(nblk):
            Lb = lpool.tile([P, C * BW], fp32, tag="L")
            tt = small.tile([P, BW], mybir.dt.int32, tag="t")
            # one DMA per class per sweep, alternating HWDGE rings
            for c in range(C):
                eng = nc.sync if c % 2 == 0 else nc.scalar
                eng.dma_start(Lb[:, c * BW : (c + 1) * BW], src_log[c, blk])
            nc.sync.dma_start(tt[:], src_tgt[blk, :, :, 0].opt())

            for s in range(nsub):
                if on("act"):
                    E = epool.tile([P, M17 * T], fp32, tag="E")
                    E3 = E[:].rearrange("p (j s) -> p j s", s=M17)  # [p,T,M17]
                if on("recip"):
                    R = rpool.tile([P, M17 * T], fp32, tag="R")
                    R3 = R[:].rearrange("p (j s) -> p j s", s=M17)
                if on("gp"):
                    zt = zpool.tile([P, 8 * T], fp32, tag="zt")
                    z3 = zt[:].rearrange("p (j s) -> p j s", s=8)   # [p,T,8]

                # class-major view of this sub-iteration's slice of Lb
                Ljc = Lb[:].rearrange("p (c j) -> p j c", c=C)[
                    :, s * T : (s + 1) * T, :
                ]  # [p, T, C]
                ts = tt[:, s * T : (s + 1) * T]

                # e = exp(logits); ACT converts class-major -> chunk-major
                if on("act"):
                    nc.scalar.activation(
                        E3[:, :, 0:C], Ljc, mybir.ActivationFunctionType.Exp
                    )

                # Z = sum_c e, binary tree. Levels 1-2 on GPSIMD (runs
                # concurrently; all DVE ops here are 1x so no port clash).
                if on("gp"):
                    nc.gpsimd.tensor_tensor(
                        z3[:, :, 0:8], E3[:, :, 0:8], E3[:, :, 8:16], AL.add
                    )
                    nc.gpsimd.tensor_tensor(
                        z3[:, :, 0:4], z3[:, :, 0:4], z3[:, :, 4:8], AL.add
                    )
                # DVE tail, split to stay under the drain knee
                if on("dvez"):
                    nsp = max(1, T // 128)
                    for sp in range(nsp):
                        js = slice(sp * (T // nsp), (sp + 1) * (T // nsp))
                        nc.vector.tensor_tensor(
                            z3[:, js, 0:2], z3[:, js, 0:2], z3[:, js, 2:4], AL.add
                        )
                    for sp in range(nsp):
                        js = slice(sp * (T // nsp), (sp + 1) * (T // nsp))
                        nc.vector.tensor_tensor(
                            E3[:, js, C], z3[:, js, 0], z3[:, js, 1], AL.add
                        )

                # r = 1/Z -> slot 16 of R
                if on("recip"):
                    if fast_recip:
                        nc.vector.reciprocal_approx_fast(R3[:, :, C], E3[:, :, C])
                    else:
                        scr = small.tile([P, T], fp32, tag="scr")
                        nc.vector.reciprocal_approx_accurate(
                            R3[:, :, C], E3[:, :, C], scr[:]
                        )

                # mr[:, :, c] = (t == c) * r  (DVE, one small op per class)
                if on("stt"):
                    for c in range(C):
                        nc.vector.scalar_tensor_tensor(
                            R3[:, :, c],
                            ts,
                            float(c),
                            R3[:, :, C],
                            op0=AL.is_equal,
                            op1=AL.mult,
                        )

                # stats matmuls: contract over partitions, G chunks packed
                # per matmul via contiguous [p, g*17] operand slices
                if on("mm"):
                    groups = [(m * G, min(G, T - m * G)) for m in range(nmm)]
                    # start/stop matmuls must cover the full PSUM region:
                    # keep full-size groups first and last
                    if groups[-1][1] != G and len(groups) >= 2:
                        groups[-1], groups[-2] = groups[-2], groups[-1]
                    for m, (g0, g) in enumerate(groups):
                        first = blk == 0 and s == 0 and m == 0
                        last = blk == nblk - 1 and s == nsub - 1 and m == nmm - 1
                        nc.tensor.matmul(
                            acc[0 : g * M17, 0 : g * M17],
                            E[:, g0 * M17 : (g0 + g) * M17],
                            R[:, g0 * M17 : (g0 + g) * M17],
                            start=first,
                            stop=last,
                        )
        outs = fin.tile([MOUT, MOUT], fp32)
        if on("mm"):
            nc.vector.tensor_copy(outs[:], acc[:])
        else:
            nc.vector.memset(outs[:], 0.0)
        nc.sync.dma_start(out_d.ap(), outs[:])

    # per-partition byte budgets keep pools inside SBUF for any shape
    budget = 196 * 1024
    lbufs = 2
    sbufs = 2
    budget -= lbufs * C * BW * 4 + sbufs * BW * 4
    esz, rsz, zsz = M17 * T * 4, M17 * T * 4, 8 * T * 4
    ebufs = max(1, min(4, int(budget * 0.40) // esz))
    rbufs = max(1, min(3, int(budget * 0.35) // rsz))
    zbufs = max(1, min(3, int(budget * 0.20) // zsz))
    with tile.TileContext(nc) as tc:
        with (
            tc.tile_pool(name="lpool", bufs=lbufs) as lpool,
            tc.tile_pool(name="epool", bufs=ebufs) as epool,
            tc.tile_pool(name="rpool", bufs=rbufs) as rpool,
            tc.tile_pool(name="zpool", bufs=zbufs) as zpool,
            tc.tile_pool(name="small", bufs=sbufs) as small,
            tc.tile_pool(name="psum", bufs=1, space="PSUM") as psump,
            tc.tile_pool(name="fin", bufs=1) as fin,
        ):
            pools = (lpool, epool, rpool, zpool, small, psump, fin)
            if loop_reps > 1:
                with tc.For_i(0, loop_reps, 1, hint_engines=(mybir.EngineType.PE,)):
                    body(tc, pools)
            else:
                body(tc, pools)
    nc.compile()
    return nc


_NC_CACHE = {}


def _get_nc():
    if "nc" not in _NC_CACHE:
        _NC_CACHE["nc"] = build()
    return _NC_CACHE["nc"]


def stats_from_out(out_mat):
    """Sum the G diagonal 17x17 blocks -> one 17x17 stats matrix."""
    S = np.zeros((M17, M17), np.float64)
    for g in range(G):
        S += out_mat[g * M17 : (g + 1) * M17, g * M17 : (g + 1) * M17].astype(
            np.float64
        )
    return S


def loss_from_stats(S_per_b):
    """S_per_b: (B, 17, 17) combined stats -> scalar loss (reference formula)."""
    idx = np.arange(C)
    inter = S_per_b[:, idx, idx]          # (B, C)
    probs_sum = S_per_b[:, 0:C, C]        # (B, C)
    counts = S_per_b[:, C, 0:C]           # (B, C)
    dice = (2.0 * inter + SMOOTH) / (probs_sum + counts + SMOOTH)
    mask = np.ones(C)
    mask[IGNORE_INDEX] = 0.0
    mean_dice = (dice * mask[None, :]).sum() / (B * (C - 1))
    return np.float32(1.0 - mean_dice)


def shard_inputs(logits, targets):
    """Core i gets batch i//4, d-slab i%4."""
    in_maps = []
    for i in range(NCORES):
        b, q = divmod(i, 4)
        lg = np.ascontiguousarray(
            logits[b, :, q * DSH : (q + 1) * DSH]
        ).reshape(C, N)
        tg = (
            np.ascontiguousarray(targets[b, q * DSH : (q + 1) * DSH])
            .reshape(N)
            .astype(np.int64, copy=False)
            .view(np.int32)
        )
        in_maps.append({"logits": lg, "targets": tg})
    return in_maps


def kernel(logits, targets):
    logits = np.asarray(logits)
    targets = np.asarray(targets)
    nc = _get_nc()
    in_maps = shard_inputs(logits, targets)
    res = run_bass_kernel_spmd(nc, in_maps, list(range(NCORES))).results
    S = np.zeros((B, M17, M17), np.float64)
    for i in range(NCORES):
        S[i // 4] += stats_from_out(res[i]["out"])
    return loss_from_stats(S)



# revision 4
# speedup vs baseline: 1.7958x; 1.7958x over previous
"""Memory-efficient Dice loss on 8 Trainium2 NeuronCores.

Full inputs:
  logits  (2, 16, 64, 128, 128) fp32
  targets (2, 64, 128, 128) int64  (values 0..15)
Output: scalar fp32 loss = 1 - mean_{b, c != 0} dice[b, c].

Sharding: 8 cores over (B=2) x (D quartered into 4 slabs of 4).
Each core reduces its shard to one 119x119 stats matrix; host combines
the tiny per-core stats and applies the dice formula.

Per-core math (voxels n, classes c), fp16 on-chip:
  e[n,c]   = exp(logit[n,c])            (|logit| < ~6, no max-sub)
  Z[n]     = sum_c e[n,c]   (binary tree adds)
  r[n]     = 1/Z[n]
  R[n,c']  = (t[n] == c') * r[n]
  Stats via PSUM-accumulated fp16 matmuls contracting over voxels:
    lhsT slots = [e (16) | Z],  rhs slots = [R (16) | r]
    out[c,c'] diag = intersection, out[c,16] = probs_sum, out[16,c'] = counts

Layout: "blocked chunk-major". E/R tiles hold element (chunk m, slot c,
lane g) at m*119 + c*7 + g, so each of the 74 matmuls per block reads a
CONTIGUOUS 119-column slice (walrus requires 1-free-dim matmul
operands), while every elementwise op still sees packed 7-element fp16
runs - which keeps the DVE 2x (tensor_tensor) and 4x (tensor_scalar_ptr)
perf modes live.

Padding: each block carries T=518 voxel columns = 512 real + 6 host-side
pads (7 | 518). Pad logits are 0 (e=1, Z=16, r=1/16 - all finite/exact),
pad target is 100.0 (matches no class), so pads contribute ONLY
r*e = 1/16 per class to probs_sum; the host subtracts that exactly.

DMA: the host pre-permutes each core's logits shard to [nblk][p][c][j]
fp32 (and targets to [nblk][p][j] fp16), so each block is ONE fully
contiguous dma_start. This replaces the baseline's 68 strided DMAs and
262k-descriptor int32-pair targets gather that kept all 16 DMA engines
~66% busy.
"""

import numpy as np

import concourse.bass as bass
import concourse.mybir as mybir
import concourse.tile as tile
from concourse import bacc
from concourse.bass_utils import run_bass_kernel_spmd

B, C, D, H, W = 2, 16, 64, 128, 128
P = 128            # SBUF partitions
NCORES = 8
DSH = D // 4       # d-planes per core
N = DSH * H * W    # voxels per core = 262144
M17 = C + 1        # 17 = classes + (Z | r) slot
G = 7              # packed chunk lanes per matmul
MOUT = G * M17     # 119

NBLK = 4
TR = N // (P * NBLK)        # real voxel columns per block = 512
NMM = (TR + G - 1) // G     # matmuls per block = 74
T = NMM * G                 # padded columns per block = 518
NPAD = T - TR               # 6 pad columns per partition per block

SMOOTH = 1.0
IGNORE_INDEX = 0
PAD_TARGET = 100.0          # matches no class


def build():
    """Build the SPMD single-core Bass program."""
    fp32 = mybir.dt.float32
    fp16 = mybir.dt.float16
    AL = mybir.AluOpType

    nc = bacc.Bacc("TRN2", target_bir_lowering=False, debug=False)
    logits_d = nc.dram_tensor("logits", [NBLK, P, C * T], fp32, kind="ExternalInput")
    targets_d = nc.dram_tensor("targets", [NBLK, P, T], fp16, kind="ExternalInput")
    out_d = nc.dram_tensor("out", [MOUT, MOUT], fp32, kind="ExternalOutput")

    def body(tc, pools):
        lpool, tpool, epool, rpool, zpool, fpool, psump, fin = pools
        acc = psump.tile([MOUT, MOUT], fp32)
        for blk in range(NBLK):
            Lb = lpool.tile([P, C * T], fp32, tag="L")
            tt = tpool.tile([P, T], fp16, tag="t")
            nc.sync.dma_start(Lb[:], logits_d.ap()[blk])
            nc.sync.dma_start(tt[:], targets_d.ap()[blk])

            E = epool.tile([P, NMM * MOUT], fp16, tag="E")
            R = rpool.tile([P, NMM * MOUT], fp16, tag="R")
            zt = zpool.tile([P, 8 * T], fp16, tag="zt")
            Zf = fpool.tile([P, T], fp32, tag="Zf")
            Rf = fpool.tile([P, T], fp32, tag="Rf")
            # blocked views: [p, chunk m, slot c, lane g]
            E4 = E[:].rearrange("p (m c g) -> p m c g", m=NMM, c=M17)
            R4 = R[:].rearrange("p (m c g) -> p m c g", m=NMM, c=M17)
            L3 = Lb[:].rearrange("p (c j) -> p c j", c=C)      # [p, 16, T]
            Lg = Lb[:].rearrange("p (c m g) -> p c m g", c=C, g=G)
            z3 = zt[:].rearrange("p (s j) -> p s j", s=8)      # [p, 8, T]
            zg = zt[:].rearrange("p (s m g) -> p s m g", s=8, g=G)

            # e = exp(logits): class-major contiguous reads, 7-lane writes
            nc.scalar.activation(
                E4[:, :, 0:C, :].rearrange("p m c g -> p c m g"),
                Lg[:],
                mybir.ActivationFunctionType.Exp,
            )

            # Z = sum_c e: tree. L1 on DVE (2x fp16), L2/L3 on GpSimd,
            # tail on DVE (writes both fp32 Z and the fp16 Z slot).
            nc.vector.tensor_tensor(
                zg[:],
                E4[:, :, 0:8, :].rearrange("p m s g -> p s m g"),
                E4[:, :, 8:16, :].rearrange("p m s g -> p s m g"),
                AL.add,
            )
            nc.gpsimd.tensor_tensor(
                z3[:, 0:4, :], z3[:, 0:4, :], z3[:, 4:8, :], AL.add
            )
            nc.gpsimd.tensor_tensor(
                z3[:, 0:2, :], z3[:, 0:2, :], z3[:, 2:4, :], AL.add
            )
            nc.vector.tensor_tensor(Zf[:], z3[:, 0, :], z3[:, 1, :], AL.add)
            nc.vector.tensor_tensor(
                E4[:, :, C, :],
                z3[:, 0, :],
                z3[:, 1, :],
                AL.add,
            )

            # r = 1/Z (fp32 approx), then fp16 copy into R slot 16
            nc.vector.reciprocal_approx_fast(Rf[:], Zf[:])
            nc.vector.tensor_copy(
                R4[:, :, C, :], Rf[:]
            )

            # R[:, c] = (t == c) * r   (DVE 4x: all operands fp16 SBUF)
            rslot = R4[:, :, C, :]
            for c in range(C):
                nc.vector.scalar_tensor_tensor(
                    R4[:, :, c, :],
                    tt[:],
                    float(c),
                    rslot,
                    op0=AL.is_equal,
                    op1=AL.mult,
                )

            # stats matmuls: contiguous 119-column operands, fp16
            for m in range(NMM):
                nc.tensor.matmul(
                    acc[:],
                    E[:, m * MOUT : (m + 1) * MOUT],
                    R[:, m * MOUT : (m + 1) * MOUT],
                    start=(blk == 0 and m == 0),
                    stop=(blk == NBLK - 1 and m == NMM - 1),
                )
        outs = fin.tile([MOUT, MOUT], fp32)
        nc.vector.tensor_copy(outs[:], acc[:])
        nc.sync.dma_start(out_d.ap(), outs[:])

    with tile.TileContext(nc) as tc:
        with (
            tc.tile_pool(name="lpool", bufs=2) as lpool,
            tc.tile_pool(name="tpool", bufs=2) as tpool,
            tc.tile_pool(name="epool", bufs=2) as epool,
            tc.tile_pool(name="rpool", bufs=2) as rpool,
            tc.tile_pool(name="zpool", bufs=2) as zpool,
            tc.tile_pool(name="fpool", bufs=2) as fpool,
            tc.tile_pool(name="psum", bufs=1, space="PSUM") as psump,
            tc.tile_pool(name="fin", bufs=1) as fin,
        ):
            body(tc, (lpool, tpool, epool, rpool, zpool, fpool, psump, fin))
    nc.compile()
    return nc


_NC_CACHE = {}


def _get_nc():
    if "nc" not in _NC_CACHE:
        _NC_CACHE["nc"] = build()
    return _NC_CACHE["nc"]


def stats_from_out(out_mat):
    """out[c1*7+g, c2*7+g] summed over g -> one 17x17 stats matrix."""
    M = out_mat.astype(np.float64).reshape(M17, G, M17, G)
    return np.einsum("agbg->ab", M)


def loss_from_stats(S_per_b):
    """S_per_b: (B, 17, 17) combined stats -> scalar loss."""
    idx = np.arange(C)
    inter = S_per_b[:, idx, idx]          # (B, C)
    probs_sum = S_per_b[:, 0:C, C]        # (B, C)
    counts = S_per_b[:, C, 0:C]           # (B, C)
    dice = (2.0 * inter + SMOOTH) / (probs_sum + counts + SMOOTH)
    mask = np.ones(C)
    mask[IGNORE_INDEX] = 0.0
    mean_dice = (dice * mask[None, :]).sum() / (B * (C - 1))
    return np.float32(1.0 - mean_dice)


def shard_inputs(logits, targets):
    """Core i gets batch i//4, d-slab i%4.

    Device layout (voxel n = p*(NBLK*TR) + blk*TR + j for j < TR):
      logits  [NBLK, P, C, T] fp32  (cols TR..T zero-padded)
      targets [NBLK, P, T]    fp16  (pad = PAD_TARGET)
    """
    in_maps = []
    for i in range(NCORES):
        b, q = divmod(i, 4)
        lg = logits[b, :, q * DSH : (q + 1) * DSH].reshape(C, P, NBLK, TR)
        lgp = np.zeros((NBLK, P, C, T), np.float32)
        lgp[:, :, :, 0:TR] = lg.transpose(2, 1, 0, 3)
        tg = targets[b, q * DSH : (q + 1) * DSH].reshape(P, NBLK, TR)
        tgp = np.full((NBLK, P, T), PAD_TARGET, np.float16)
        tgp[:, :, 0:TR] = tg.transpose(1, 0, 2)
        in_maps.append({"logits": lgp.reshape(NBLK, P, C * T), "targets": tgp})
    return in_maps


def kernel(logits, targets):
    logits = np.asarray(logits)
    targets = np.asarray(targets)
    nc = _get_nc()
    in_maps = shard_inputs(logits, targets)
    res = run_bass_kernel_spmd(nc, in_maps, list(range(NCORES))).results
    S = np.zeros((B, M17, M17), np.float64)
    for i in range(NCORES):
        S[i // 4] += stats_from_out(res[i]["out"])
    # pad voxels (logit 0 -> e=1, r=1/16) add exactly 1/16 per class to
    # probs_sum; masks are all-zero (PAD_TARGET matches no class)
    npad_per_batch = NPAD * P * NBLK * 4
    S[:, 0:C, C] -= npad_per_batch / 16.0
    return loss_from_stats(S)


# revision 5
# speedup vs baseline: 2.1617x; 1.2037x over previous
"""Memory-efficient Dice loss on 8 Trainium2 NeuronCores.

Full inputs:
  logits  (2, 16, 64, 128, 128) fp32
  targets (2, 64, 128, 128) int64  (values 0..15)
Output: scalar fp32 loss = 1 - mean_{b, c != 0} dice[b, c].

Sharding: 8 cores over (B=2) x (D quartered into 4 slabs of 16).
Each core reduces its shard to one 128x128 stats matrix; host combines
the tiny per-core stats and applies the dice formula.

Per-core math (voxels n, classes c), fp16 on-chip:
  e[n,c]   = exp(logit[n,c])            (|logit| < ~6, no max-sub)
  Z[n]     = sum_c e[n,c]   (binary tree adds)
  r[n]     = 1/Z[n]
  R[n,c']  = (t[n] == c') * r[n]
  Stats via PSUM-accumulated fp16 matmuls contracting over voxels:
    S16[c1,c2] = sum_n e[n,c1] * R[n,c2]
  Because the one-hot masks partition unity (every voxel has exactly one
  target class), S16 already contains ALL the stats:
    diag(S16)      = intersection
    S16.sum(cols)  = sum_n e_c1 * r = probs_sum
    S16.sum(rows)  = sum_n (Z*r) * mask_c2 = counts   (Z*r == 1)
  No dedicated Z/r slots -> 16 slots x 8 lanes = full 128-wide matmuls
  and T=512 divides exactly (no padding).

Layout: "blocked chunk-major". E/R tiles hold element (chunk m, slot c,
lane g) at m*128 + c*8 + g, so each of the 64 matmuls per block reads a
CONTIGUOUS 128-column slice (walrus requires 1-free-dim matmul
operands), while every elementwise op sees packed, 16-byte-aligned
8-element fp16 runs - keeping the DVE 2x/4x perf modes live.

DMA: the host pre-permutes each core's logits shard to [nblk][p][c][j]
fp32 (and targets to [nblk][p][j] fp16), so each block is ONE fully
contiguous dma_start.
"""

import numpy as np

import concourse.bass as bass
import concourse.mybir as mybir
import concourse.tile as tile
from concourse import bacc
from concourse.bass_utils import run_bass_kernel_spmd

B, C, D, H, W = 2, 16, 64, 128, 128
P = 128            # SBUF partitions
NCORES = 8
DSH = D // 4       # d-planes per core
N = DSH * H * W    # voxels per core = 262144
G = 8              # packed chunk lanes per matmul
MOUT = C * G       # 128

NBLK = 4
T = N // (P * NBLK)         # voxel columns per block = 512
NMM = T // G                # matmuls per block = 64

SMOOTH = 1.0
IGNORE_INDEX = 0


def build():
    """Build the SPMD single-core Bass program."""
    fp32 = mybir.dt.float32
    fp16 = mybir.dt.float16
    AL = mybir.AluOpType

    nc = bacc.Bacc("TRN2", target_bir_lowering=False, debug=False)
    logits_d = nc.dram_tensor("logits", [NBLK, P, C * T], fp32, kind="ExternalInput")
    targets_d = nc.dram_tensor("targets", [NBLK, P, T], fp16, kind="ExternalInput")
    out_d = nc.dram_tensor("out", [MOUT, MOUT], fp32, kind="ExternalOutput")

    def body(tc, pools):
        lpool, tpool, epool, rpool, zpool, fpool, psump, fin = pools
        acc = psump.tile([MOUT, MOUT], fp32)
        for blk in range(NBLK):
            Lb = lpool.tile([P, C * T], fp32, tag="L")
            tt = tpool.tile([P, T], fp16, tag="t")
            nc.sync.dma_start(Lb[:], logits_d.ap()[blk])
            nc.sync.dma_start(tt[:], targets_d.ap()[blk])

            E = epool.tile([P, NMM * MOUT], fp16, tag="E")
            R = rpool.tile([P, NMM * MOUT], fp16, tag="R")
            zt = zpool.tile([P, 8 * T], fp16, tag="zt")
            Zf = fpool.tile([P, T], fp32, tag="Zf")
            Rf = fpool.tile([P, T], fp32, tag="Rf")
            rc = fpool.tile([P, T], fp16, tag="rc")
            # blocked views: [p, chunk m, slot c, lane g]
            E4 = E[:].rearrange("p (m c g) -> p m c g", m=NMM, c=C)
            R4 = R[:].rearrange("p (m c g) -> p m c g", m=NMM, c=C)
            Lg = Lb[:].rearrange("p (c m g) -> p c m g", c=C, g=G)
            z3 = zt[:].rearrange("p (s j) -> p s j", s=8)      # [p, 8, T]
            zg = zt[:].rearrange("p (s m g) -> p s m g", s=8, g=G)

            # e = exp(logits): class-major contiguous reads, 8-lane writes
            nc.scalar.activation(
                E4[:].rearrange("p m c g -> p c m g"),
                Lg[:],
                mybir.ActivationFunctionType.Exp,
            )

            # Z = sum_c e: tree. L1 on DVE (2x fp16), L2/L3 on GpSimd,
            # fp32 tail on DVE.
            nc.vector.tensor_tensor(
                zg[:],
                E4[:, :, 0:8, :].rearrange("p m s g -> p s m g"),
                E4[:, :, 8:16, :].rearrange("p m s g -> p s m g"),
                AL.add,
            )
            nc.gpsimd.tensor_tensor(
                z3[:, 0:4, :], z3[:, 0:4, :], z3[:, 4:8, :], AL.add
            )
            nc.gpsimd.tensor_tensor(
                z3[:, 0:2, :], z3[:, 0:2, :], z3[:, 2:4, :], AL.add
            )
            nc.vector.tensor_tensor(Zf[:], z3[:, 0, :], z3[:, 1, :], AL.add)

            # r = 1/Z (fp32 approx), then contiguous fp16 copy
            nc.vector.reciprocal_approx_fast(Rf[:], Zf[:])
            nc.vector.tensor_copy(rc[:], Rf[:])

            # R[:, c] = (t == c) * r   (DVE 4x: all operands fp16 SBUF)
            for c in range(C):
                nc.vector.scalar_tensor_tensor(
                    R4[:, :, c, :],
                    tt[:],
                    float(c),
                    rc[:],
                    op0=AL.is_equal,
                    op1=AL.mult,
                )

            # stats matmuls: contiguous 128-column operands, fp16
            for m in range(NMM):
                nc.tensor.matmul(
                    acc[:],
                    E[:, m * MOUT : (m + 1) * MOUT],
                    R[:, m * MOUT : (m + 1) * MOUT],
                    start=(blk == 0 and m == 0),
                    stop=(blk == NBLK - 1 and m == NMM - 1),
                )
        outs = fin.tile([MOUT, MOUT], fp32)
        nc.vector.tensor_copy(outs[:], acc[:])
        nc.sync.dma_start(out_d.ap(), outs[:])

    with tile.TileContext(nc) as tc:
        with (
            tc.tile_pool(name="lpool", bufs=2) as lpool,
            tc.tile_pool(name="tpool", bufs=2) as tpool,
            tc.tile_pool(name="epool", bufs=2) as epool,
            tc.tile_pool(name="rpool", bufs=2) as rpool,
            tc.tile_pool(name="zpool", bufs=2) as zpool,
            tc.tile_pool(name="fpool", bufs=2) as fpool,
            tc.tile_pool(name="psum", bufs=1, space="PSUM") as psump,
            tc.tile_pool(name="fin", bufs=1) as fin,
        ):
            body(tc, (lpool, tpool, epool, rpool, zpool, fpool, psump, fin))
    nc.compile()
    return nc


_NC_CACHE = {}


def _get_nc():
    if "nc" not in _NC_CACHE:
        _NC_CACHE["nc"] = build()
    return _NC_CACHE["nc"]


def stats_from_out(out_mat):
    """out[c1*8+g, c2*8+g] summed over g -> one 16x16 stats matrix."""
    M = out_mat.astype(np.float64).reshape(C, G, C, G)
    return np.einsum("agbg->ab", M)


def loss_from_stats(S_per_b):
    """S_per_b: (B, 16, 16) combined stats -> scalar loss."""
    idx = np.arange(C)
    inter = S_per_b[:, idx, idx]          # (B, C)
    probs_sum = S_per_b.sum(axis=2)       # (B, C)  sum_n e_c * r
    counts = S_per_b.sum(axis=1)          # (B, C)  sum_n Z*r*mask_c
    dice = (2.0 * inter + SMOOTH) / (probs_sum + counts + SMOOTH)
    mask = np.ones(C)
    mask[IGNORE_INDEX] = 0.0
    mean_dice = (dice * mask[None, :]).sum() / (B * (C - 1))
    return np.float32(1.0 - mean_dice)


def shard_inputs(logits, targets):
    """Core i gets batch i//4, d-slab i%4.

    Device layout (voxel n = p*(NBLK*T) + blk*T + j):
      logits  [NBLK, P, C, T] fp32
      targets [NBLK, P, T]    fp16
    """
    in_maps = []
    for i in range(NCORES):
        b, q = divmod(i, 4)
        lg = logits[b, :, q * DSH : (q + 1) * DSH].reshape(C, P, NBLK, T)
        lg = np.ascontiguousarray(lg.transpose(2, 1, 0, 3), dtype=np.float32)
        tg = targets[b, q * DSH : (q + 1) * DSH].reshape(P, NBLK, T)
        tg = np.ascontiguousarray(tg.transpose(1, 0, 2)).astype(np.float16)
        in_maps.append({"logits": lg.reshape(NBLK, P, C * T), "targets": tg})
    return in_maps


def kernel(logits, targets):
    logits = np.asarray(logits)
    targets = np.asarray(targets)
    nc = _get_nc()
    in_maps = shard_inputs(logits, targets)
    res = run_bass_kernel_spmd(nc, in_maps, list(range(NCORES))).results
    S = np.zeros((B, C, C), np.float64)
    for i in range(NCORES):
        S[i // 4] += stats_from_out(res[i]["out"])
    return loss_from_stats(S)


# revision 9
# speedup vs baseline: 2.3497x; 1.0870x over previous
"""Memory-efficient Dice loss on 8 Trainium2 NeuronCores.

Full inputs:
  logits  (2, 16, 64, 128, 128) fp32
  targets (2, 64, 128, 128) int64  (values 0..15)
Output: scalar fp32 loss = 1 - mean_{b, c != 0} dice[b, c].

Sharding: 8 cores over (B=2) x (D quartered into 4 slabs of 16).
Each core reduces its shard to one 128x128 stats matrix; host combines
the tiny per-core stats and applies the dice formula.

Per-core math (voxels n, classes c), fp16 on-chip:
  e[n,c]   = exp(logit[n,c])            (|logit| < ~6, no max-sub)
  Z[n]     = sum_c e[n,c]   (binary tree adds)
  r[n]     = 1/Z[n]
  R[n,c']  = (t[n] == c') * r[n]
  Stats via PSUM-accumulated fp16 matmuls contracting over voxels:
    S16[c1,c2] = sum_n e[n,c1] * R[n,c2]
  Because the one-hot masks partition unity (every voxel has exactly one
  target class), S16 already contains ALL the stats:
    diag(S16)      = intersection
    S16.sum(cols)  = sum_n e_c1 * r = probs_sum
    S16.sum(rows)  = sum_n (Z*r) * mask_c2 = counts   (Z*r == 1)
  No dedicated Z/r slots -> 16 slots x 8 lanes = full 128-wide matmuls
  and T=512 divides exactly (no padding).

Layout: "blocked chunk-major". E/R tiles hold element (chunk m, slot c,
lane g) at m*128 + c*8 + g, so each of the 64 matmuls per block reads a
CONTIGUOUS 128-column slice (walrus requires 1-free-dim matmul
operands), while every elementwise op sees packed, 16-byte-aligned
8-element fp16 runs - keeping the DVE 2x/4x perf modes live.

DMA: the host pre-permutes each core's logits shard to [nblk][p][c][j]
fp32 (and targets to [nblk][p][j] fp16), so each block is ONE fully
contiguous dma_start.
"""

import numpy as np

import concourse.bass as bass
import concourse.mybir as mybir
import concourse.tile as tile
from concourse import bacc
from concourse.bass_utils import run_bass_kernel_spmd

B, C, D, H, W = 2, 16, 64, 128, 128
P = 128            # SBUF partitions
NCORES = 8
DSH = D // 4       # d-planes per core
N = DSH * H * W    # voxels per core = 262144
G = 8              # packed chunk lanes per matmul
MOUT = C * G       # 128

NBLK = 4
T = N // (P * NBLK)         # voxel columns per block = 512
NMM = T // G                # matmuls per block = 64

SMOOTH = 1.0
IGNORE_INDEX = 0
NDVE_STT = 16      # stt classes on DVE (gpsimd lacks TensorScalarPtr)


def build():
    """Build the SPMD single-core Bass program."""
    fp32 = mybir.dt.float32
    fp16 = mybir.dt.float16
    AL = mybir.AluOpType

    nc = bacc.Bacc("TRN2", target_bir_lowering=False, debug=False)
    logits_d = nc.dram_tensor("logits", [NBLK, P, C * T], fp32, kind="ExternalInput")
    targets_d = nc.dram_tensor("targets", [NBLK, P, T], fp16, kind="ExternalInput")
    out_d = nc.dram_tensor("out", [MOUT, MOUT], fp32, kind="ExternalOutput")

    def body(tc, pools):
        lpool, tpool, epool, rpool, zpool, fpool, psump, fin = pools
        acc = psump.tile([MOUT, MOUT], fp32)
        for blk in range(NBLK):
            Lb = lpool.tile([P, C * T], fp32, tag="L")
            tt = tpool.tile([P, T], fp16, tag="t")
            ldma = nc.sync if blk % 2 == 0 else nc.gpsimd
            ldma.dma_start(Lb[:], logits_d.ap()[blk])
            nc.sync.dma_start(tt[:], targets_d.ap()[blk])

            E = epool.tile([P, NMM * MOUT], fp16, tag="E")
            R = rpool.tile([P, NMM * MOUT], fp16, tag="R")
            zt = zpool.tile([P, 8 * T], fp16, tag="zt")
            Zf = fpool.tile([P, T], fp32, tag="Zf")
            Rf = fpool.tile([P, T], fp32, tag="Rf")
            rc = fpool.tile([P, T], fp16, tag="rc")
            # blocked views: [p, chunk m, slot c, lane g]
            E4 = E[:].rearrange("p (m c g) -> p m c g", m=NMM, c=C)
            R4 = R[:].rearrange("p (m c g) -> p m c g", m=NMM, c=C)
            Lg = Lb[:].rearrange("p (c m g) -> p c m g", c=C, g=G)
            z3 = zt[:].rearrange("p (s j) -> p s j", s=8)      # [p, 8, T]
            zg = zt[:].rearrange("p (s m g) -> p s m g", s=8, g=G)

            # e = exp(logits): class-major contiguous reads, 8-lane writes
            nc.scalar.activation(
                E4[:].rearrange("p m c g -> p c m g"),
                Lg[:],
                mybir.ActivationFunctionType.Exp,
            )

            # Z = sum_c e: full tree on DVE (2x fp16) so DVE never stalls
            # mid-chain waiting on another engine; fp32 tail for recip.
            nc.vector.tensor_tensor(
                zg[:],
                E4[:, :, 0:8, :].rearrange("p m s g -> p s m g"),
                E4[:, :, 8:16, :].rearrange("p m s g -> p s m g"),
                AL.add,
            )
            nc.vector.tensor_tensor(
                z3[:, 0:4, :], z3[:, 0:4, :], z3[:, 4:8, :], AL.add
            )
            nc.vector.tensor_tensor(
                z3[:, 0:2, :], z3[:, 0:2, :], z3[:, 2:4, :], AL.add
            )
            nc.vector.tensor_tensor(Zf[:], z3[:, 0, :], z3[:, 1, :], AL.add)

            # r = 1/Z (fp32 approx), then contiguous fp16 copy
            nc.vector.reciprocal_approx_fast(Rf[:], Zf[:])
            nc.vector.tensor_copy(rc[:], Rf[:])

            # R[:, c] = (t == c) * r; DVE classes first (they gate the
            # first matmuls), GpSimd tail overlaps the next block's DVE
            for c in range(C):
                eng = nc.vector if c < NDVE_STT else nc.gpsimd
                eng.scalar_tensor_tensor(
                    R4[:, :, c, :],
                    tt[:],
                    float(c),
                    rc[:],
                    op0=AL.is_equal,
                    op1=AL.mult,
                )

            # stats matmuls: contiguous 128-column operands, fp16
            for m in range(NMM):
                nc.tensor.matmul(
                    acc[:],
                    E[:, m * MOUT : (m + 1) * MOUT],
                    R[:, m * MOUT : (m + 1) * MOUT],
                    start=(blk == 0 and m == 0),
                    stop=(blk == NBLK - 1 and m == NMM - 1),
                )
        outs = fin.tile([MOUT, MOUT], fp32)
        nc.vector.tensor_copy(outs[:], acc[:])
        nc.sync.dma_start(out_d.ap(), outs[:])
        # --- perf probes (results unused; diagnose DVE fast modes) ---
        pa = fin.tile([P, T], fp16, tag="pa")
        pb = fin.tile([P, T], fp16, tag="pb")
        nc.vector.memset(pb[:], 1.0)  # probe0: memset fp16 contig
        nc.vector.scalar_tensor_tensor(
            pa[:], pb[:], 3.0, pb[:], op0=AL.is_equal, op1=AL.mult
        )  # probe1: stt fully contiguous
        nc.vector.tensor_tensor(pa[:], pb[:], pb[:], AL.is_equal)  # probe2: TT is_eq contig
        nc.vector.tensor_tensor(pa[:], pb[:], pb[:], AL.mult)  # probe3: TT mult contig
        nc.vector.tensor_copy(pa[:], pb[:])  # probe4: copy fp16 contig

    with tile.TileContext(nc) as tc:
        with (
            tc.tile_pool(name="lpool", bufs=2) as lpool,
            tc.tile_pool(name="tpool", bufs=2) as tpool,
            tc.tile_pool(name="epool", bufs=2) as epool,
            tc.tile_pool(name="rpool", bufs=2) as rpool,
            tc.tile_pool(name="zpool", bufs=2) as zpool,
            tc.tile_pool(name="fpool", bufs=2) as fpool,
            tc.tile_pool(name="psum", bufs=1, space="PSUM") as psump,
            tc.tile_pool(name="fin", bufs=1) as fin,
        ):
            body(tc, (lpool, tpool, epool, rpool, zpool, fpool, psump, fin))
    nc.compile()
    return nc


_NC_CACHE = {}


def _get_nc():
    if "nc" not in _NC_CACHE:
        _NC_CACHE["nc"] = build()
    return _NC_CACHE["nc"]


def stats_from_out(out_mat):
    """out[c1*8+g, c2*8+g] summed over g -> one 16x16 stats matrix."""
    M = out_mat.astype(np.float64).reshape(C, G, C, G)
    return np.einsum("agbg->ab", M)


def loss_from_stats(S_per_b):
    """S_per_b: (B, 16, 16) combined stats -> scalar loss."""
    idx = np.arange(C)
    inter = S_per_b[:, idx, idx]          # (B, C)
    probs_sum = S_per_b.sum(axis=2)       # (B, C)  sum_n e_c * r
    counts = S_per_b.sum(axis=1)          # (B, C)  sum_n Z*r*mask_c
    dice = (2.0 * inter + SMOOTH) / (probs_sum + counts + SMOOTH)
    mask = np.ones(C)
    mask[IGNORE_INDEX] = 0.0
    mean_dice = (dice * mask[None, :]).sum() / (B * (C - 1))
    return np.float32(1.0 - mean_dice)


def shard_inputs(logits, targets):
    """Core i gets batch i//4, d-slab i%4.

    Device layout (voxel n = p*(NBLK*T) + blk*T + j):
      logits  [NBLK, P, C, T] fp32
      targets [NBLK, P, T]    fp16
    """
    in_maps = []
    for i in range(NCORES):
        b, q = divmod(i, 4)
        lg = logits[b, :, q * DSH : (q + 1) * DSH].reshape(C, P, NBLK, T)
        lg = np.ascontiguousarray(lg.transpose(2, 1, 0, 3), dtype=np.float32)
        tg = targets[b, q * DSH : (q + 1) * DSH].reshape(P, NBLK, T)
        tg = np.ascontiguousarray(tg.transpose(1, 0, 2)).astype(np.float16)
        in_maps.append({"logits": lg.reshape(NBLK, P, C * T), "targets": tg})
    return in_maps


def kernel(logits, targets):
    logits = np.asarray(logits)
    targets = np.asarray(targets)
    nc = _get_nc()
    in_maps = shard_inputs(logits, targets)
    res = run_bass_kernel_spmd(nc, in_maps, list(range(NCORES))).results
    S = np.zeros((B, C, C), np.float64)
    for i in range(NCORES):
        S[i // 4] += stats_from_out(res[i]["out"])
    return loss_from_stats(S)


# revision 12
# speedup vs baseline: 2.5193x; 1.0722x over previous
"""Memory-efficient Dice loss on 8 Trainium2 NeuronCores.

Full inputs:
  logits  (2, 16, 64, 128, 128) fp32
  targets (2, 64, 128, 128) int64  (values 0..15)
Output: scalar fp32 loss = 1 - mean_{b, c != 0} dice[b, c].

Sharding: 8 cores over (B=2) x (D quartered into 4 slabs of 16).
Each core reduces its shard to one 128x128 stats matrix; host combines
the tiny per-core stats and applies the dice formula.

Per-core math (voxels n, classes c), fp16 on-chip:
  e[n,c]   = exp(logit[n,c])            (|logit| < ~6, no max-sub)
  Z[n]     = sum_c e[n,c]   (binary tree adds)
  r[n]     = 1/Z[n]
  R[n,c']  = (t[n] == c') * r[n]
  Stats via PSUM-accumulated fp16 matmuls contracting over voxels:
    S16[c1,c2] = sum_n e[n,c1] * R[n,c2]
  Because the one-hot masks partition unity (every voxel has exactly one
  target class), S16 already contains ALL the stats:
    diag(S16)      = intersection
    S16.sum(cols)  = sum_n e_c1 * r = probs_sum
    S16.sum(rows)  = sum_n (Z*r) * mask_c2 = counts   (Z*r == 1)
  No dedicated Z/r slots -> 16 slots x 8 lanes = full 128-wide matmuls
  and T=512 divides exactly (no padding).

Layout: "blocked chunk-major". E/R tiles hold element (chunk m, slot c,
lane g) at m*128 + c*8 + g, so each of the 64 matmuls per block reads a
CONTIGUOUS 128-column slice (walrus requires 1-free-dim matmul
operands), while every elementwise op sees packed, 16-byte-aligned
8-element fp16 runs - keeping the DVE 2x/4x perf modes live.

DMA: the host pre-permutes each core's logits shard to [nblk][p][c][j]
fp32 (and targets to [nblk][p][j] fp16), so each block is ONE fully
contiguous dma_start.
"""

import numpy as np

import concourse.bass as bass
import concourse.mybir as mybir
import concourse.tile as tile
from concourse import bacc
from concourse.bass_utils import run_bass_kernel_spmd

B, C, D, H, W = 2, 16, 64, 128, 128
P = 128            # SBUF partitions
NCORES = 8
DSH = D // 4       # d-planes per core
N = DSH * H * W    # voxels per core = 262144
G = 8              # packed chunk lanes per matmul
MOUT = C * G       # 128

NBLK = 4
T = N // (P * NBLK)         # voxel columns per block = 512
NMM = T // G                # matmuls per block = 64

SMOOTH = 1.0
IGNORE_INDEX = 0
NDVE_STT = 16      # stt classes on DVE (gpsimd lacks TensorScalarPtr)
HC = C // 2        # classes per DMA/EXP half


def build():
    """Build the SPMD single-core Bass program."""
    fp32 = mybir.dt.float32
    fp16 = mybir.dt.float16
    AL = mybir.AluOpType

    nc = bacc.Bacc("TRN2", target_bir_lowering=False, debug=False)
    logits_d = nc.dram_tensor("logits", [NBLK, P, C * T], fp32, kind="ExternalInput")
    targets_d = nc.dram_tensor("targets", [NBLK, P, T], fp16, kind="ExternalInput")
    out_d = nc.dram_tensor("out", [MOUT, MOUT], fp32, kind="ExternalOutput")

    def body(tc, pools):
        lpool, tpool, epool, rpool, zpool, fpool, psump, fin = pools
        acc = psump.tile([MOUT, MOUT], fp32)
        for blk in range(NBLK):
            Lb = lpool.tile([P, C * T], fp32, tag="L")
            tt = tpool.tile([P, T], fp16, tag="t")
            # split each block's logits across both DMA rings (sync /
            # gpsimd) so neither ring paces the pipeline; targets ride
            # the ring with the lighter half
            ring_a = nc.sync if blk % 2 == 0 else nc.gpsimd
            ring_b = nc.gpsimd if blk % 2 == 0 else nc.sync
            la = logits_d.ap()[blk]
            ring_a.dma_start(Lb[:, 0 : HC * T], la[:, 0 : HC * T])
            ring_b.dma_start(Lb[:, HC * T : C * T], la[:, HC * T : C * T])
            ring_a.dma_start(tt[:], targets_d.ap()[blk])

            E = epool.tile([P, NMM * MOUT], fp16, tag="E")
            R = rpool.tile([P, NMM * MOUT], fp16, tag="R")
            zt = zpool.tile([P, 8 * T], fp16, tag="zt")
            Zf = fpool.tile([P, T], fp32, tag="Zf")
            Rf = fpool.tile([P, T], fp32, tag="Rf")
            rc = fpool.tile([P, T], fp16, tag="rc")
            # blocked views: [p, chunk m, slot c, lane g]
            E4 = E[:].rearrange("p (m c g) -> p m c g", m=NMM, c=C)
            R4 = R[:].rearrange("p (m c g) -> p m c g", m=NMM, c=C)
            Lg = Lb[:].rearrange("p (c m g) -> p c m g", c=C, g=G)
            z3 = zt[:].rearrange("p (s j) -> p s j", s=8)      # [p, 8, T]
            zg = zt[:].rearrange("p (s m g) -> p s m g", s=8, g=G)

            # e = exp(logits): one op per class-half so each starts as
            # soon as its half-DMA lands
            nc.scalar.activation(
                E4[:, :, 0:HC, :].rearrange("p m c g -> p c m g"),
                Lg[:, 0:HC],
                mybir.ActivationFunctionType.Exp,
            )
            nc.scalar.activation(
                E4[:, :, HC:C, :].rearrange("p m c g -> p c m g"),
                Lg[:, HC:C],
                mybir.ActivationFunctionType.Exp,
            )

            # Z = sum_c e: full tree on DVE (2x fp16) so DVE never stalls
            # mid-chain; L1 is split per class-half, fp32 tail for recip.
            nc.vector.tensor_tensor(
                zg[:, 0:4],
                E4[:, :, 0:4, :].rearrange("p m s g -> p s m g"),
                E4[:, :, 4:8, :].rearrange("p m s g -> p s m g"),
                AL.add,
            )
            nc.vector.tensor_tensor(
                zg[:, 4:8],
                E4[:, :, 8:12, :].rearrange("p m s g -> p s m g"),
                E4[:, :, 12:16, :].rearrange("p m s g -> p s m g"),
                AL.add,
            )
            nc.vector.tensor_tensor(
                z3[:, 0:4, :], z3[:, 0:4, :], z3[:, 4:8, :], AL.add
            )
            nc.vector.tensor_tensor(
                z3[:, 0:2, :], z3[:, 0:2, :], z3[:, 2:4, :], AL.add
            )
            nc.vector.tensor_tensor(Zf[:], z3[:, 0, :], z3[:, 1, :], AL.add)

            # r = 1/Z (fp32 approx), then contiguous fp16 copy
            nc.vector.reciprocal_approx_fast(Rf[:], Zf[:])
            nc.vector.tensor_copy(rc[:], Rf[:])

            # R[:, c] = (t == c) * r; DVE classes first (they gate the
            # first matmuls), GpSimd tail overlaps the next block's DVE
            for c in range(C):
                eng = nc.vector if c < NDVE_STT else nc.gpsimd
                eng.scalar_tensor_tensor(
                    R4[:, :, c, :],
                    tt[:],
                    float(c),
                    rc[:],
                    op0=AL.is_equal,
                    op1=AL.mult,
                )

            # stats matmuls: contiguous 128-column operands, fp16
            for m in range(NMM):
                nc.tensor.matmul(
                    acc[:],
                    E[:, m * MOUT : (m + 1) * MOUT],
                    R[:, m * MOUT : (m + 1) * MOUT],
                    start=(blk == 0 and m == 0),
                    stop=(blk == NBLK - 1 and m == NMM - 1),
                )
        outs = fin.tile([MOUT, MOUT], fp32)
        nc.vector.tensor_copy(outs[:], acc[:])
        nc.sync.dma_start(out_d.ap(), outs[:])
        # --- perf probes (results unused; diagnose DVE fast modes) ---
        pa = fin.tile([P, 2 * T], fp16, tag="pa")
        pb = fin.tile([P, 2 * T], fp16, tag="pb")
        pc = fin.tile([P, T], fp32, tag="pc")
        pd = fin.tile([P, C * T], fp16, tag="pd")
        nc.vector.memset(pb[:], 1.0)
        nc.vector.memset(pc[:], 1.0)
        nc.vector.scalar_tensor_tensor(
            pa[:], pb[:], 3.0, pb[:], op0=AL.is_equal, op1=AL.mult
        )  # probeA: stt contiguous 1024 elems (2x->~870ns, 1x->~1130ns)
        nc.vector.scalar_tensor_tensor(
            pa[:, 0:T], pb[:, 0:T], 3.0, pc[:], op0=AL.is_equal, op1=AL.mult
        )  # probeB: stt 512 with fp32 in1
        pbv = pb[:].rearrange("p (o j) -> p o j", o=2)[:, 0:1, :]
        pdv = pd[:].rearrange("p (c j) -> p c j", c=C)
        pin, pout = bass.broadcast_tensor_aps(pbv, pdv)
        nc.vector.tensor_copy(pdv, pin)  # probeC: bcast copy 16x512

    with tile.TileContext(nc) as tc:
        with (
            tc.tile_pool(name="lpool", bufs=2) as lpool,
            tc.tile_pool(name="tpool", bufs=2) as tpool,
            tc.tile_pool(name="epool", bufs=2) as epool,
            tc.tile_pool(name="rpool", bufs=2) as rpool,
            tc.tile_pool(name="zpool", bufs=2) as zpool,
            tc.tile_pool(name="fpool", bufs=2) as fpool,
            tc.tile_pool(name="psum", bufs=1, space="PSUM") as psump,
            tc.tile_pool(name="fin", bufs=1) as fin,
        ):
            body(tc, (lpool, tpool, epool, rpool, zpool, fpool, psump, fin))
    nc.compile()
    return nc


_NC_CACHE = {}


def _get_nc():
    if "nc" not in _NC_CACHE:
        _NC_CACHE["nc"] = build()
    return _NC_CACHE["nc"]


def stats_from_out(out_mat):
    """out[c1*8+g, c2*8+g] summed over g -> one 16x16 stats matrix."""
    M = out_mat.astype(np.float64).reshape(C, G, C, G)
    return np.einsum("agbg->ab", M)


def loss_from_stats(S_per_b):
    """S_per_b: (B, 16, 16) combined stats -> scalar loss."""
    idx = np.arange(C)
    inter = S_per_b[:, idx, idx]          # (B, C)
    probs_sum = S_per_b.sum(axis=2)       # (B, C)  sum_n e_c * r
    counts = S_per_b.sum(axis=1)          # (B, C)  sum_n Z*r*mask_c
    dice = (2.0 * inter + SMOOTH) / (probs_sum + counts + SMOOTH)
    mask = np.ones(C)
    mask[IGNORE_INDEX] = 0.0
    mean_dice = (dice * mask[None, :]).sum() / (B * (C - 1))
    return np.float32(1.0 - mean_dice)


def shard_inputs(logits, targets):
    """Core i gets batch i//4, d-slab i%4.

    Device layout (voxel n = p*(NBLK*T) + blk*T + j):
      logits  [NBLK, P, C, T] fp32
      targets [NBLK, P, T]    fp16
    """
    in_maps = []
    for i in range(NCORES):
        b, q = divmod(i, 4)
        lg = logits[b, :, q * DSH : (q + 1) * DSH].reshape(C, P, NBLK, T)
        lg = np.ascontiguousarray(lg.transpose(2, 1, 0, 3), dtype=np.float32)
        tg = targets[b, q * DSH : (q + 1) * DSH].reshape(P, NBLK, T)
        tg = np.ascontiguousarray(tg.transpose(1, 0, 2)).astype(np.float16)
        in_maps.append({"logits": lg.reshape(NBLK, P, C * T), "targets": tg})
    return in_maps


def kernel(logits, targets):
    logits = np.asarray(logits)
    targets = np.asarray(targets)
    nc = _get_nc()
    in_maps = shard_inputs(logits, targets)
    res = run_bass_kernel_spmd(nc, in_maps, list(range(NCORES))).results
    S = np.zeros((B, C, C), np.float64)
    for i in range(NCORES):
        S[i // 4] += stats_from_out(res[i]["out"])
    return loss_from_stats(S)
